# revision 14
# baseline (speedup 1.0000x reference)
"""Bass/Tile kernel for nn_BasicGRUClassifier on 8 Trainium2 NeuronCores.

Strategy (data-parallel over batch, 32 samples/core, bf16 matmul datapath):
  All on-chip tensors use [H=128 partitions, B=32 free] layout, t-major
  PSUM gate banks (col = tl*32 + b) so every critical-path access is
  contiguous.

  Per chunk of LCH=16 timesteps, PSUM holds the pre-activation gates:
    RU0 [128,1024] (2 banks, double-buffered): L0 r at 0:512, u at 512:1024
    RU1 [128,1024] (2 banks, single buffer):   L1 r / u
    O0  [128,512]  (1 bank):  L0 o            O1 [128,512] (1 bank): L1 o
  Banks are seeded by the batched x-projection matmuls (L0 biases ride a
  ones-channel appended to X's last K-tile; L1 r/u biases are K=1 matmuls
  against a ones row; L1 o bias comes in through the tanh bias operand).
  The recurrent U@h matmuls accumulate into per-step 32-col slices, so no
  identity-prefill matmuls and no PSUM->SBUF gate copies are needed.

  Cell update is restructured as
    m = (u-1)*h          (off critical path)
    e = u*o
    h' = e - m           (= (1-u)h + u*o)
  so little work separates tanh from the next step's matmuls.

  Everything the PE touches is bf16 (fp32 matmuls double-pump the PE:
  2x LDWEIGHTS + 2x MATMUL per instruction). PSUM accumulation stays
  fp32; activations read fp32 PSUM and emit bf16. Verified numerically:
  bf16 end-to-end rel err vs fp32 reference = 3.9e-3 (tolerance 2e-2).
"""

import numpy as np
import ml_dtypes

HID = 128
IN_CH = 271
SEQ = 281
NCLS = 1854
BATCH = 256
NCORES = 8
BL = BATCH // NCORES  # 32 per-core batch
LCH = 16              # timesteps per chunk
G3 = 3 * HID
RUW = 2 * LCH * BL    # 1024: r/u region stride inside a RU tile

_CACHE = {}


def _build(seq_t):
    import concourse.bacc as bacc
    import concourse.tile as tile
    import concourse.mybir as mybir
    from contextlib import ExitStack

    fp32 = mybir.dt.float32
    bf16 = mybir.dt.bfloat16
    AF = mybir.ActivationFunctionType
    ALU = mybir.AluOpType

    nch = (seq_t + LCH - 1) // LCH
    chlen = [min(LCH, seq_t - c * LCH) for c in range(nch)]
    UOFF = LCH * BL  # 512: u-region offset (bank B of a RU tile)

    nc = bacc.Bacc()
    XT = nc.dram_tensor("XT", [IN_CH, seq_t * BL], bf16, kind="ExternalInput")
    WX0 = nc.dram_tensor("WX0", [IN_CH + 1, G3], bf16, kind="ExternalInput")
    UH0 = nc.dram_tensor("UH0", [HID, G3], bf16, kind="ExternalInput")
    WX1 = nc.dram_tensor("WX1", [HID, G3], bf16, kind="ExternalInput")
    UH1 = nc.dram_tensor("UH1", [HID, G3], bf16, kind="ExternalInput")
    B1R = nc.dram_tensor("B1R", [1, G3], bf16, kind="ExternalInput")
    B1O = nc.dram_tensor("B1O", [HID, 1], fp32, kind="ExternalInput")
    WFC = nc.dram_tensor("WFC", [HID, NCLS], bf16, kind="ExternalInput")
    BFC = nc.dram_tensor("BFC", [1, NCLS], bf16, kind="ExternalInput")
    OUT = nc.dram_tensor("OUT", [BL, NCLS], fp32, kind="ExternalOutput")

    ksz = [128, 128, IN_CH - 256 + 1]  # third tile: 15 channels + ones row

    with tile.TileContext(nc) as tc:
        with ExitStack() as ctx:
            const = ctx.enter_context(tc.tile_pool(name="const", bufs=1))
            seqp = ctx.enter_context(tc.tile_pool(name="seqp", bufs=2))
            cellp = ctx.enter_context(tc.tile_pool(name="cellp", bufs=4))
            outp = ctx.enter_context(tc.tile_pool(name="outp", bufs=1))
            ru0ps = ctx.enter_context(tc.tile_pool(name="ru0ps", bufs=2, space="PSUM"))
            ru1ps = ctx.enter_context(tc.tile_pool(name="ru1ps", bufs=1, space="PSUM"))
            o0ps = ctx.enter_context(tc.tile_pool(name="o0ps", bufs=1, space="PSUM"))
            o1ps = ctx.enter_context(tc.tile_pool(name="o1ps", bufs=1, space="PSUM"))

            # ---- constants into SBUF ----
            xt_sb = []
            for k in range(3):
                t_ = const.tile([ksz[k], seq_t * BL], bf16, tag=f"xt{k}")
                c0 = sum(ksz[:k])
                if k < 2:
                    nc.sync.dma_start(out=t_, in_=XT[c0:c0 + ksz[k], :])
                else:
                    # row 15 is the ones-channel that carries the L0 biases;
                    # fill the tile with 1.0, then overlay the 15 real rows
                    nc.vector.memset(t_, 1.0)
                    nc.sync.dma_start(out=t_[0:15, :], in_=XT[256:271, :])
                xt_sb.append(t_)
            wx0_sb = []
            for k in range(3):
                t_ = const.tile([ksz[k], G3], bf16, tag=f"wx0{k}")
                c0 = sum(ksz[:k])
                nc.sync.dma_start(out=t_, in_=WX0[c0:c0 + ksz[k], :])
                wx0_sb.append(t_)
            uh0_sb = const.tile([HID, G3], bf16, tag="uh0")
            nc.sync.dma_start(out=uh0_sb, in_=UH0[:, :])
            wx1_sb = const.tile([HID, G3], bf16, tag="wx1")
            nc.sync.dma_start(out=wx1_sb, in_=WX1[:, :])
            uh1_sb = const.tile([HID, G3], bf16, tag="uh1")
            nc.sync.dma_start(out=uh1_sb, in_=UH1[:, :])
            b1_sb = const.tile([1, G3], bf16, tag="b1")
            nc.sync.dma_start(out=b1_sb, in_=B1R[:, :])
            b1o_sb = const.tile([HID, 1], fp32, tag="b1o")
            nc.sync.dma_start(out=b1o_sb, in_=B1O[:, :])
            wfc_sb = const.tile([HID, NCLS], bf16, tag="wfc")
            nc.sync.dma_start(out=wfc_sb, in_=WFC[:, :])
            bfc_sb = const.tile([1, NCLS], bf16, tag="bfc")
            nc.sync.dma_start(out=bfc_sb, in_=BFC[:, :])
            ones_sb = const.tile([1, LCH * BL], bf16, tag="ones")
            nc.vector.memset(ones_sb, 1.0)
            h0i = const.tile([HID, BL], bf16, tag="h0i")
            nc.vector.memset(h0i, 0.0)
            h1i = const.tile([HID, BL], bf16, tag="h1i")
            nc.vector.memset(h1i, 0.0)

            ru0_bank = {}
            ru1_bank = {}
            o0_bank = {}
            o1_bank = {}
            h0seq = {}

            # ---- PE warm-up ----
            # The HAM clock gate starts at K=4/8 (1.2 GHz) and only lifts to
            # 2.4 GHz after ~3.4us of sustained PE activity; the recurrence
            # alone never trips it. Burn ~5us of dummy matmuls up front; the
            # steady-state PE gaps are well under the ~3.4us re-throttle
            # window, so the array stays warm afterwards.
            warm = ru1ps.tile([HID, RUW], fp32, tag="ru1")
            for i in range(24):
                nc.tensor.matmul(warm[:, 0:512], b1_sb[:, 0:HID],
                                 ones_sb[:, 0:512], start=True, stop=True)

            # Deferrable phase matmuls are queued as closures and drained a
            # couple per cell, so they fill PE idle gaps mid-chunk instead of
            # serializing at chunk boundaries ahead of critical cell matmuls.
            pending = []

            def drain_pending(k=2):
                for _ in range(min(k, len(pending))):
                    pending.pop(0)()

            def phase_l0_ru(c, defer=True):
                """Chunk c's L0 r/u pre-activations: batched x-projection
                (biases ride the ones-channel in xt_sb[2])."""
                n = chlen[c] * BL
                t0 = c * LCH * BL
                ru = ru0ps.tile([HID, RUW], fp32, tag="ru0")
                ru0_bank[c] = ru
                for g, off in ((0, 0), (1, UOFF)):
                    for k in range(3):
                        def mm(g=g, off=off, k=k):
                            nc.tensor.matmul(
                                ru[:, off:off + n],
                                wx0_sb[k][:, g * HID:(g + 1) * HID],
                                xt_sb[k][:, t0:t0 + n],
                                start=(k == 0), stop=False)
                        if defer:
                            pending.append(mm)
                        else:
                            mm()

            def phase_l0_o(c):
                """Chunk c's L0 o pre-activation (single-buffered bank, so
                emitted at the start of chunk c, not prefetched)."""
                n = chlen[c] * BL
                t0 = c * LCH * BL
                ob = o0ps.tile([HID, LCH * BL], fp32, tag="o0")
                o0_bank[c] = ob
                for k in range(3):
                    nc.tensor.matmul(
                        ob[:, 0:n], wx0_sb[k][:, 2 * HID:G3],
                        xt_sb[k][:, t0:t0 + n], start=(k == 0), stop=False)

            def phase_l1(c):
                """L1 bias + x-projection for chunk c from completed h0seq.
                Queued (not emitted inline) so the matmuls spread into the
                next chunk's PE gaps."""
                n = chlen[c] * BL
                ru = ru1ps.tile([HID, RUW], fp32, tag="ru1")
                ob = o1ps.tile([HID, LCH * BL], fp32, tag="o1")
                ru1_bank[c] = ru
                o1_bank[c] = ob
                hs = h0seq[c]
                mms = [
                    lambda: nc.tensor.matmul(
                        ru[:, 0:n], b1_sb[:, 0:HID], ones_sb[:, 0:n],
                        start=True, stop=False),
                    lambda: nc.tensor.matmul(
                        ru[:, UOFF:UOFF + n], b1_sb[:, HID:2 * HID],
                        ones_sb[:, 0:n], start=True, stop=False),
                    lambda: nc.tensor.matmul(
                        ru[:, 0:n], wx1_sb[:, 0:HID], hs[:, 0:n],
                        start=False, stop=False),
                    lambda: nc.tensor.matmul(
                        ru[:, UOFF:UOFF + n], wx1_sb[:, HID:2 * HID],
                        hs[:, 0:n], start=False, stop=False),
                    lambda: nc.tensor.matmul(
                        ob[:, 0:n], wx1_sb[:, 2 * HID:G3], hs[:, 0:n],
                        start=True, stop=False),
                ]
                pending.extend(mms)

            def cell(layer, c, tl, h_prev, h_out):
                """One GRU cell; returns AP of the new state (== h_out)."""
                if layer == 0:
                    ru_bank, ob, uh = ru0_bank[c], o0_bank[c], uh0_sb
                else:
                    ru_bank, ob, uh = ru1_bank[c], o1_bank[c], uh1_sb
                s = tl * BL
                nc.tensor.matmul(ru_bank[:, s:s + BL], uh[:, 0:HID], h_prev,
                                 start=False, stop=True)
                nc.tensor.matmul(ru_bank[:, UOFF + s:UOFF + s + BL],
                                 uh[:, HID:2 * HID], h_prev,
                                 start=False, stop=True)
                # one deferred phase matmul here: it executes inside the
                # sigmoid+rh latency window while the PE would sit idle
                drain_pending(1)
                # layer1's plain elementwise ops ride on GpSimd to keep the
                # DVE free for layer0's critical chain
                ew = nc.vector if layer == 0 else nc.gpsimd
                ru_t = cellp.tile([HID, 2 * BL], bf16, tag=f"ru{layer}t")
                nc.scalar.activation(
                    ru_t.rearrange("p (g x) -> p g x", g=2),
                    ru_bank.rearrange("p (g x) -> p g x", g=2)[:, :, s:s + BL],
                    AF.Sigmoid)
                rh = cellp.tile([HID, BL], bf16, tag=f"rh{layer}")
                ew.tensor_mul(rh, ru_t[:, 0:BL], h_prev)
                m = cellp.tile([HID, BL], bf16, tag=f"m{layer}")
                nc.vector.scalar_tensor_tensor(
                    m, ru_t[:, BL:2 * BL], 1.0, h_prev,
                    op0=ALU.subtract, op1=ALU.mult)
                nc.tensor.matmul(ob[:, s:s + BL], uh[:, 2 * HID:G3], rh,
                                 start=False, stop=True)
                # another deferred phase matmul inside the tanh+e+h' window
                drain_pending(1)
                o_t = cellp.tile([HID, BL], bf16, tag=f"o{layer}")
                if layer == 0:
                    nc.scalar.activation(o_t, ob[:, s:s + BL], AF.Tanh)
                else:
                    nc.scalar.activation(o_t, ob[:, s:s + BL], AF.Tanh,
                                         bias=b1o_sb[:, 0:1])
                e = cellp.tile([HID, BL], bf16, tag=f"e{layer}")
                ew.tensor_mul(e, ru_t[:, BL:2 * BL], o_t)
                ew.tensor_sub(h_out, e, m)
                return h_out

            # ---------- main pipeline ----------
            phase_l0_ru(0, defer=False)
            h0_cur = h0i[:, :]
            h1_cur = h1i[:, :]
            next_l1 = 0

            def l1_step(tg):
                nonlocal h1_cur, next_l1
                c1, tl1 = divmod(tg, LCH)
                h1_new = cellp.tile([HID, BL], bf16, tag="h1s")
                h1_cur = cell(1, c1, tl1, h1_cur, h1_new[:, :])
                next_l1 = tg + 1

            for c in range(nch):
                phase_l0_o(c)
                if c + 1 < nch:
                    phase_l0_ru(c + 1)
                hs = seqp.tile([HID, LCH * BL], bf16, tag="h0seq")
                h0seq[c] = hs
                for tl in range(chlen[c]):
                    h0_cur = cell(0, c, tl, h0_cur, hs[:, tl * BL:(tl + 1) * BL])
                    # the previous chunk's queued L1 phase matmuls must all be
                    # emitted before the first l1_step that reads their banks
                    if tl == 0:
                        drain_pending(5)
                    tg1 = c * LCH + tl - LCH
                    if tg1 >= 0:
                        l1_step(tg1)
                phase_l1(c)

            drain_pending(len(pending))
            for tg1 in range(next_l1, seq_t):
                l1_step(tg1)

            # ---------- FC ----------
            out_sb = outp.tile([BL, NCLS], fp32, tag="osb")
            nsl = [512, 512, 512, NCLS - 3 * 512]
            for i in range(4):
                n0 = i * 512
                fc = ru1ps.tile([BL, 512], fp32, tag="ru1")
                pf = fc[:, 0:nsl[i]]
                nc.tensor.matmul(pf, ones_sb[:, 0:BL], bfc_sb[:, n0:n0 + nsl[i]],
                                 start=True, stop=False)
                nc.tensor.matmul(pf, h1_cur, wfc_sb[:, n0:n0 + nsl[i]],
                                 start=False, stop=True)
                nc.scalar.activation(out_sb[:, n0:n0 + nsl[i]], pf, AF.Identity)
            nc.sync.dma_start(out=OUT[:, :], in_=out_sb)

    nc.finalize()
    return nc


def _prep_consts(inputs):
    bf = ml_dtypes.bfloat16
    # L0 x-weights with the bias row appended (matches the ones-channel
    # appended to X's last K-tile)
    Wx0 = np.ascontiguousarray(np.concatenate([
        np.concatenate([inputs["Wr0"][:IN_CH], inputs["Wu0"][:IN_CH],
                        inputs["Wo0"][:IN_CH]], axis=1),
        np.concatenate([inputs["br0"], inputs["bu0"], inputs["bo0"]])[None, :],
    ], axis=0).astype(bf))
    Uh0 = np.ascontiguousarray(np.concatenate(
        [inputs["Wr0"][IN_CH:], inputs["Wu0"][IN_CH:], inputs["Wo0"][IN_CH:]],
        axis=1).astype(bf))
    Wx1 = np.ascontiguousarray(np.concatenate(
        [inputs["Wr1"][:HID], inputs["Wu1"][:HID], inputs["Wo1"][:HID]],
        axis=1).astype(bf))
    Uh1 = np.ascontiguousarray(np.concatenate(
        [inputs["Wr1"][HID:], inputs["Wu1"][HID:], inputs["Wo1"][HID:]],
        axis=1).astype(bf))
    B1R = np.ascontiguousarray(np.concatenate(
        [inputs["br1"], inputs["bu1"], inputs["bo1"]])[None, :].astype(bf))
    B1O = np.ascontiguousarray(inputs["bo1"][:, None].astype(np.float32))
    WFC = np.ascontiguousarray(inputs["Wfc"].astype(bf))
    BFC = np.ascontiguousarray(inputs["bfc"][None, :].astype(bf))
    return dict(WX0=Wx0, UH0=Uh0, WX1=Wx1, UH1=Uh1, B1R=B1R, B1O=B1O,
                WFC=WFC, BFC=BFC)


def kernel(_trace=False, **inputs):
    from concourse.bass_utils import run_bass_kernel_spmd

    seq_t = inputs["X"].shape[2]
    if "nc" not in _CACHE or _CACHE.get("seq_t") != seq_t:
        _CACHE["nc"] = _build(seq_t)
        _CACHE["seq_t"] = seq_t
    nc = _CACHE["nc"]

    consts = _prep_consts(inputs)
    bf = ml_dtypes.bfloat16
    # [B, C, T] -> per-core [C, T, BL] (t-major columns: col = t*BL + b)
    X = inputs["X"].astype(bf)
    in_maps = []
    for c in range(NCORES):
        m = dict(consts)
        xc = X[c * BL:(c + 1) * BL].transpose(1, 2, 0)  # [C, T, BL]
        m["XT"] = np.ascontiguousarray(xc).reshape(IN_CH, seq_t * BL)
        in_maps.append(m)

    res = run_bass_kernel_spmd(nc, in_maps, core_ids=list(range(NCORES)),
                               trace=_trace)
    out = np.concatenate([r["OUT"] for r in res.results], axis=0)
    if _trace:
        _CACHE["last_exec_time_ns"] = res.exec_time_ns
        _CACHE["last_profile"] = res.profile_json
    return out


# revision 17
# speedup vs baseline: 1.2236x; 1.2236x over previous
"""Bass/Tile kernel for nn_BasicGRUClassifier on 8 Trainium2 NeuronCores.

Strategy (data-parallel over batch, 32 samples/core, bf16 matmul datapath,
cross-layer fusion):

  The two GRU layers are software-pipelined with a fixed lag of LCH=8
  steps and FUSED: one "slot" (c, tl) advances layer0 at t = c*8+tl and
  layer1 at t-8 with SINGLE activation/vector instructions over paired
  [128, 2, 32] / [128, 64] operands. Fusion halves the per-step
  instruction count and semaphore traffic on the serial critical path,
  which is what bounds this latency-dominated recurrence.

  PSUM layout per chunk c (shared banks make the fused APs single-tile):
    RUP(c) [128,1024] (2 banks, bufs=3):
       bank A: r0(c) cols 0:256   | r1(c-1) cols 256:512
       bank B: u0(c) cols 512:768 | u1(c-1) cols 768:1024
    OP(c)  [128,512] (1 bank, bufs=2): o0(c) 0:256 | o1(c-1) 256:512
  Banks are seeded by batched x-projection matmuls (L0 biases ride a
  ones-channel appended to X's last K-tile; L1 biases are K=1 matmuls
  against a ones row).

  The state update h' = (1-u)h + u*o is decomposed as m = (u-1)h,
  e = u*o, and the next step's gate pre-activations are accumulated as
  x + (-U)@m + U@e directly in PSUM (pre-negated weight copies), so the
  only work between tanh and the next step's matmuls is one vector op
  (e). h' = e - m itself is computed off the critical path (GpSimd) into
  paired state tiles: hpair(c) slot tl holds [h0(t) | h1(t-8)], written
  by one op and consumed as one operand by the next slot.

  Everything the PE touches is bf16 (fp32 matmuls double-pump the PE);
  PSUM accumulation stays fp32; activations read fp32 PSUM and emit
  bf16. bf16 end-to-end rel err vs fp32 reference = 4.1e-3 (tol 2e-2).
"""

import numpy as np
import ml_dtypes

HID = 128
IN_CH = 271
SEQ = 281
NCLS = 1854
BATCH = 256
NCORES = 8
BL = BATCH // NCORES  # 32 per-core batch
LCH = 8               # timesteps per chunk == layer pipeline lag
G3 = 3 * HID
CW = LCH * BL         # 256: one gate region width
R0, R1, U0, U1 = 0, CW, 2 * CW, 3 * CW
O0, O1 = 0, CW

_CACHE = {}


def _build(seq_t):
    import concourse.bacc as bacc
    import concourse.tile as tile
    import concourse.mybir as mybir
    from contextlib import ExitStack

    fp32 = mybir.dt.float32
    bf16 = mybir.dt.bfloat16
    AF = mybir.ActivationFunctionType
    ALU = mybir.AluOpType

    nch = (seq_t + LCH - 1) // LCH
    chlen = [min(LCH, seq_t - c * LCH) for c in range(nch)]

    nc = bacc.Bacc()
    XT = nc.dram_tensor("XT", [IN_CH, seq_t * BL], bf16, kind="ExternalInput")
    WX0 = nc.dram_tensor("WX0", [IN_CH + 1, G3], bf16, kind="ExternalInput")
    UH0 = nc.dram_tensor("UH0", [HID, G3], bf16, kind="ExternalInput")
    UN0 = nc.dram_tensor("UN0", [HID, 2 * HID], bf16, kind="ExternalInput")
    WX1 = nc.dram_tensor("WX1", [HID, G3], bf16, kind="ExternalInput")
    UH1 = nc.dram_tensor("UH1", [HID, G3], bf16, kind="ExternalInput")
    UN1 = nc.dram_tensor("UN1", [HID, 2 * HID], bf16, kind="ExternalInput")
    B1R = nc.dram_tensor("B1R", [1, G3], bf16, kind="ExternalInput")
    WFC = nc.dram_tensor("WFC", [HID, NCLS], bf16, kind="ExternalInput")
    BFC = nc.dram_tensor("BFC", [1, NCLS], bf16, kind="ExternalInput")
    OUT = nc.dram_tensor("OUT", [BL, NCLS], fp32, kind="ExternalOutput")

    ksz = [128, 128, IN_CH - 256 + 1]  # third tile: 15 channels + ones row

    with tile.TileContext(nc) as tc:
        with ExitStack() as ctx:
            const = ctx.enter_context(tc.tile_pool(name="const", bufs=1))
            hps = ctx.enter_context(tc.tile_pool(name="hps", bufs=2))
            cellp = ctx.enter_context(tc.tile_pool(name="cellp", bufs=4))
            outp = ctx.enter_context(tc.tile_pool(name="outp", bufs=1))
            rup = ctx.enter_context(tc.tile_pool(name="rup", bufs=3, space="PSUM"))
            opp = ctx.enter_context(tc.tile_pool(name="opp", bufs=2, space="PSUM"))

            # ---- constants into SBUF ----
            xt_sb = []
            for k in range(3):
                t_ = const.tile([ksz[k], seq_t * BL], bf16, tag=f"xt{k}")
                c0 = sum(ksz[:k])
                if k < 2:
                    nc.sync.dma_start(out=t_, in_=XT[c0:c0 + ksz[k], :])
                else:
                    # row 15 is the ones-channel carrying the L0 biases
                    nc.vector.memset(t_, 1.0)
                    nc.sync.dma_start(out=t_[0:15, :], in_=XT[256:271, :])
                xt_sb.append(t_)
            wx0_sb = []
            for k in range(3):
                t_ = const.tile([ksz[k], G3], bf16, tag=f"wx0{k}")
                c0 = sum(ksz[:k])
                nc.sync.dma_start(out=t_, in_=WX0[c0:c0 + ksz[k], :])
                wx0_sb.append(t_)

            def sbconst(name, dram, shape, dt=bf16):
                t_ = const.tile(shape, dt, tag=name)
                nc.sync.dma_start(out=t_, in_=dram[:, :])
                return t_

            uh0_sb = sbconst("uh0", UH0, [HID, G3])
            un0_sb = sbconst("un0", UN0, [HID, 2 * HID])
            wx1_sb = sbconst("wx1", WX1, [HID, G3])
            uh1_sb = sbconst("uh1", UH1, [HID, G3])
            un1_sb = sbconst("un1", UN1, [HID, 2 * HID])
            b1_sb = sbconst("b1", B1R, [1, G3])
            wfc_sb = sbconst("wfc", WFC, [HID, NCLS])
            bfc_sb = sbconst("bfc", BFC, [1, NCLS])
            ones_sb = const.tile([1, CW], bf16, tag="ones")
            nc.vector.memset(ones_sb, 1.0)
            h0i = const.tile([HID, BL], bf16, tag="h0i")
            nc.vector.memset(h0i, 0.0)

            rupt = {}
            opt_ = {}
            hpair = {}

            pending = []

            def drain_pending(k=2):
                for _ in range(min(k, len(pending))):
                    pending.pop(0)()

            def phase_l0(c):
                """Chunk c's L0 r/u/o x-projections (queued; k==0 start=True
                clears the bank, which must precede phase_l1(c-1)'s writes)."""
                n = chlen[c] * BL
                t0 = c * LCH * BL
                ru = rup.tile([HID, 4 * CW], fp32, tag="rup")
                ob = opp.tile([HID, 2 * CW], fp32, tag="opp")
                rupt[c] = ru
                opt_[c] = ob
                for g, dst in ((0, ru), (1, ru), (2, ob)):
                    off = (R0, U0, O0)[g]
                    for k in range(3):
                        def mm(g=g, dst=dst, off=off, k=k):
                            nc.tensor.matmul(
                                dst[:, off:off + n],
                                wx0_sb[k][:, g * HID:(g + 1) * HID],
                                xt_sb[k][:, t0:t0 + n],
                                start=(k == 0), stop=False)
                        pending.append(mm)

            def phase_l1(c):
                """L1 bias + x-projection for chunk c (gates live in the
                NEXT chunk's banks), queued at the end of chunk c."""
                n = chlen[c] * BL
                first = c + 1 == nch
                if first:
                    # no L0 phase allocates chunk nch's banks; do it here and
                    # let the bias matmuls clear them
                    ru_n = rup.tile([HID, 4 * CW], fp32, tag="rup")
                    ob_n = opp.tile([HID, 2 * CW], fp32, tag="opp")
                    rupt[c + 1] = ru_n
                    opt_[c + 1] = ob_n
                ru = rupt[c + 1]
                ob = opt_[c + 1]
                hs = hpair[c]
                h0ap = hs.rearrange("p (t lx) -> p t lx", lx=2 * BL)[
                    :, 0:chlen[c], 0:BL]
                for off, wslice, dst in (
                        (R1, (0, HID), ru), (U1, (HID, 2 * HID), ru),
                        (O1, (2 * HID, G3), ob)):
                    def mmb(off=off, wslice=wslice, dst=dst):
                        nc.tensor.matmul(
                            dst[:, off:off + n], b1_sb[:, wslice[0]:wslice[1]],
                            ones_sb[:, 0:n], start=first, stop=False)
                    def mmp(off=off, wslice=wslice, dst=dst):
                        nc.tensor.matmul(
                            dst[:, off:off + n], wx1_sb[:, wslice[0]:wslice[1]],
                            h0ap, start=False, stop=False)
                    pending.append(mmb)
                    pending.append(mmp)

            def emit_next_mms(uh, un, rhs_m, rhs_e, t_next, roff, uoff, which):
                """Accumulate (-U)@m / U@e into step t_next's gate columns.
                which=0 -> the m pair (stop=False), which=1 -> e (stop=True)."""
                cn, sn = divmod(t_next, LCH)
                ru = rupt[cn] if roff == R0 else rupt[cn + 1]
                s = sn * BL
                if which == 0:
                    nc.tensor.matmul(ru[:, roff + s:roff + s + BL],
                                     un[:, 0:HID], rhs_m,
                                     start=False, stop=False)
                    nc.tensor.matmul(ru[:, uoff + s:uoff + s + BL],
                                     un[:, HID:2 * HID], rhs_m,
                                     start=False, stop=False)
                else:
                    nc.tensor.matmul(ru[:, roff + s:roff + s + BL],
                                     uh[:, 0:HID], rhs_e,
                                     start=False, stop=True)
                    nc.tensor.matmul(ru[:, uoff + s:uoff + s + BL],
                                     uh[:, HID:2 * HID], rhs_e,
                                     start=False, stop=True)

            def slot(c, tl, fused, l0_only, hp_prev, h_out):
                """One pipeline slot: layer0 at t=c*8+tl (unless tail),
                layer1 at t-8 (if fused or tail). Operand width W is 64 for
                fused slots, 32 otherwise.

                hp_prev: [128, W] AP of the previous state (pair); h_out:
                [128, W] AP to write the new state (pair)."""
                t = c * LCH + tl
                s = tl * BL
                W = 2 * BL if fused else BL
                nl = 2 if fused else 1
                ru = rupt[c]
                ob = opt_[c]
                if fused:
                    rsrc = ru.rearrange("p (g x) -> p g x", g=4)[
                        :, 0:2, s:s + BL]
                    usrc = ru.rearrange("p (g x) -> p g x", g=4)[
                        :, 2:4, s:s + BL]
                    osrc = ob.rearrange("p (g x) -> p g x", g=2)[
                        :, :, s:s + BL]
                elif l0_only:
                    rsrc = ru[:, R0 + s:R0 + s + BL]
                    usrc = ru[:, U0 + s:U0 + s + BL]
                    osrc = ob[:, O0 + s:O0 + s + BL]
                else:  # tail: layer1 only; gates live in chunk c+1's banks
                    ru = rupt[c + 1]
                    ob = opt_[c + 1]
                    rsrc = ru[:, R1 + s:R1 + s + BL]
                    usrc = ru[:, U1 + s:U1 + s + BL]
                    osrc = ob[:, O1 + s:O1 + s + BL]

                ru_r = cellp.tile([HID, W], bf16, tag="rur")
                ru_u = cellp.tile([HID, W], bf16, tag="ruu")
                if fused:
                    nc.scalar.activation(
                        ru_r.rearrange("p (l x) -> p l x", l=2), rsrc,
                        AF.Sigmoid)
                    nc.scalar.activation(
                        ru_u.rearrange("p (l x) -> p l x", l=2), usrc,
                        AF.Sigmoid)
                else:
                    nc.scalar.activation(ru_r[:, 0:BL], rsrc, AF.Sigmoid)
                    nc.scalar.activation(ru_u[:, 0:BL], usrc, AF.Sigmoid)
                rh = cellp.tile([HID, W], bf16, tag="rh")
                nc.vector.tensor_mul(rh[:, 0:W], ru_r[:, 0:W], hp_prev)
                m = cellp.tile([HID, W], bf16, tag="m")
                nc.vector.scalar_tensor_tensor(
                    m[:, 0:W], ru_u[:, 0:W], 1.0, hp_prev,
                    op0=ALU.subtract, op1=ALU.mult)
                # (-U)@m accumulation into the next step's gate columns
                if l0_only or fused:
                    if t + 1 < seq_t:
                        emit_next_mms(uh0_sb, un0_sb, m[:, 0:BL], None,
                                      t + 1, R0, U0, 0)
                if not l0_only:
                    t1 = t - LCH if fused else t
                    mo = m[:, BL:2 * BL] if fused else m[:, 0:BL]
                    if t1 + 1 < seq_t:
                        emit_next_mms(uh1_sb, un1_sb, mo, None,
                                      t1 + 1, R1, U1, 0)
                drain_pending(1)
                # o-gate matmuls
                if l0_only or fused:
                    nc.tensor.matmul(ob[:, O0 + s:O0 + s + BL],
                                     uh0_sb[:, 2 * HID:G3], rh[:, 0:BL],
                                     start=False, stop=True)
                if not l0_only:
                    rho = rh[:, BL:2 * BL] if fused else rh[:, 0:BL]
                    oo = O1 if (fused or not l0_only) else O0
                    nc.tensor.matmul(ob[:, oo + s:oo + s + BL],
                                     uh1_sb[:, 2 * HID:G3], rho,
                                     start=False, stop=True)
                o_t = cellp.tile([HID, W], bf16, tag="ot")
                if fused:
                    nc.scalar.activation(
                        o_t.rearrange("p (l x) -> p l x", l=2), osrc, AF.Tanh)
                else:
                    nc.scalar.activation(o_t[:, 0:BL], osrc, AF.Tanh)
                e = cellp.tile([HID, W], bf16, tag="e")
                nc.vector.tensor_mul(e[:, 0:W], ru_u[:, 0:W], o_t[:, 0:W])
                # U@e accumulation into the next step's gate columns
                if l0_only or fused:
                    if t + 1 < seq_t:
                        emit_next_mms(uh0_sb, un0_sb, None, e[:, 0:BL],
                                      t + 1, R0, U0, 1)
                if not l0_only:
                    t1 = t - LCH if fused else t
                    eo = e[:, BL:2 * BL] if fused else e[:, 0:BL]
                    if t1 + 1 < seq_t:
                        emit_next_mms(uh1_sb, un1_sb, None, eo,
                                      t1 + 1, R1, U1, 1)
                drain_pending(1)
                # h' = e - m, off the critical path
                nc.gpsimd.tensor_sub(h_out, e[:, 0:W], m[:, 0:W])
                return h_out

            # ---------- main pipeline ----------
            phase_l0(0)
            phase_l0(1)

            for c in range(nch):
                hp = hps.tile([HID, LCH * 2 * BL], bf16, tag="hpair")
                hpair[c] = hp
                if c == 0:
                    nc.vector.memset(hp, 0.0)
                for tl in range(chlen[c]):
                    if tl == 0:
                        # flush everything queued so far -- in particular
                        # phase_l1(c-1), whose banks this chunk's sigmas read
                        drain_pending(len(pending))
                        if c + 2 < nch:
                            phase_l0(c + 2)
                    t = c * LCH + tl
                    if tl == 0:
                        hp_prev_t = hpair[c - 1] if c > 0 else None
                        pslot = (LCH - 1) * 2 * BL
                    else:
                        hp_prev_t = hp
                        pslot = (tl - 1) * 2 * BL
                    if c == 0:
                        hp_prev = h0i[:, 0:BL] if tl == 0 else \
                            hp_prev_t[:, pslot:pslot + BL]
                        h_out = hp[:, tl * 2 * BL:tl * 2 * BL + BL]
                        slot(c, tl, False, True, hp_prev, h_out)
                    else:
                        hp_prev = hp_prev_t[:, pslot:pslot + 2 * BL]
                        h_out = hp[:, tl * 2 * BL:(tl + 1) * 2 * BL]
                        slot(c, tl, True, False, hp_prev, h_out)
                phase_l1(c)

            # ---------- tail: remaining layer1 steps ----------
            # fused slots covered layer1 through t1 = seq_t-1-LCH; the last
            # LCH steps run unfused. h1(t1-1) was written by the fused slot
            # pairing layer0 step t1-1+LCH.
            tp = seq_t - LCH - 1 + LCH  # = seq_t-1: slot of h1(seq_t-LCH-1)
            cp, tlp = divmod(tp, LCH)
            h1_cur = hpair[cp][:, tlp * 2 * BL + BL:(tlp + 1) * 2 * BL]
            for t1 in range(seq_t - LCH, seq_t):
                c1, tl1 = divmod(t1, LCH)
                if tl1 == 0:
                    drain_pending(len(pending))
                h1n = cellp.tile([HID, BL], bf16, tag="h1t")
                slot(c1, tl1, False, False, h1_cur, h1n[:, :])
                h1_cur = h1n[:, :]

            drain_pending(len(pending))

            # ---------- FC ----------
            out_sb = outp.tile([BL, NCLS], fp32, tag="osb")
            nsl = [512, 512, 512, NCLS - 3 * 512]
            for i in range(4):
                n0 = i * 512
                fc = rup.tile([BL, 512], fp32, tag="rup")
                pf = fc[:, 0:nsl[i]]
                nc.tensor.matmul(pf, ones_sb[:, 0:BL], bfc_sb[:, n0:n0 + nsl[i]],
                                 start=True, stop=False)
                nc.tensor.matmul(pf, h1_cur, wfc_sb[:, n0:n0 + nsl[i]],
                                 start=False, stop=True)
                nc.scalar.activation(out_sb[:, n0:n0 + nsl[i]], pf, AF.Identity)
            nc.sync.dma_start(out=OUT[:, :], in_=out_sb)

    nc.finalize()
    return nc


def _prep_consts(inputs):
    bf = ml_dtypes.bfloat16
    Wx0 = np.ascontiguousarray(np.concatenate([
        np.concatenate([inputs["Wr0"][:IN_CH], inputs["Wu0"][:IN_CH],
                        inputs["Wo0"][:IN_CH]], axis=1),
        np.concatenate([inputs["br0"], inputs["bu0"], inputs["bo0"]])[None, :],
    ], axis=0).astype(bf))
    Uh0 = np.concatenate(
        [inputs["Wr0"][IN_CH:], inputs["Wu0"][IN_CH:], inputs["Wo0"][IN_CH:]],
        axis=1).astype(bf)
    Uh1 = np.concatenate(
        [inputs["Wr1"][HID:], inputs["Wu1"][HID:], inputs["Wo1"][HID:]],
        axis=1).astype(bf)
    Un0 = np.ascontiguousarray(-Uh0[:, 0:2 * HID])
    Un1 = np.ascontiguousarray(-Uh1[:, 0:2 * HID])
    Wx1 = np.ascontiguousarray(np.concatenate(
        [inputs["Wr1"][:HID], inputs["Wu1"][:HID], inputs["Wo1"][:HID]],
        axis=1).astype(bf))
    B1R = np.ascontiguousarray(np.concatenate(
        [inputs["br1"], inputs["bu1"], inputs["bo1"]])[None, :].astype(bf))
    WFC = np.ascontiguousarray(inputs["Wfc"].astype(bf))
    BFC = np.ascontiguousarray(inputs["bfc"][None, :].astype(bf))
    return dict(WX0=Wx0, UH0=np.ascontiguousarray(Uh0), UN0=Un0,
                WX1=Wx1, UH1=np.ascontiguousarray(Uh1), UN1=Un1,
                B1R=B1R, WFC=WFC, BFC=BFC)


def kernel(_trace=False, **inputs):
    from concourse.bass_utils import run_bass_kernel_spmd

    seq_t = inputs["X"].shape[2]
    if "nc" not in _CACHE or _CACHE.get("seq_t") != seq_t:
        _CACHE["nc"] = _build(seq_t)
        _CACHE["seq_t"] = seq_t
    nc = _CACHE["nc"]

    consts = _prep_consts(inputs)
    bf = ml_dtypes.bfloat16
    # [B, C, T] -> per-core [C, T, BL] (t-major columns: col = t*BL + b)
    X = inputs["X"].astype(bf)
    in_maps = []
    for c in range(NCORES):
        m = dict(consts)
        xc = X[c * BL:(c + 1) * BL].transpose(1, 2, 0)  # [C, T, BL]
        m["XT"] = np.ascontiguousarray(xc).reshape(IN_CH, seq_t * BL)
        in_maps.append(m)

    res = run_bass_kernel_spmd(nc, in_maps, core_ids=list(range(NCORES)),
                               trace=_trace)
    out = np.concatenate([r["OUT"] for r in res.results], axis=0)
    if _trace:
        _CACHE["last_exec_time_ns"] = res.exec_time_ns
        _CACHE["last_profile"] = res.profile_json
    return out


# revision 23
# speedup vs baseline: 1.2745x; 1.0416x over previous
"""Bass/Tile kernel for nn_BasicGRUClassifier on 8 Trainium2 NeuronCores.

Strategy (data-parallel over batch, 32 samples/core, bf16 matmul datapath,
cross-layer fusion):

  The two GRU layers are software-pipelined with a fixed lag of LCH=8
  steps and FUSED: one "slot" (c, tl) advances layer0 at t = c*8+tl and
  layer1 at t-8 with SINGLE activation/vector instructions over paired
  [128, 2, 32] / [128, 64] operands. Fusion halves the per-step
  instruction count and semaphore traffic on the serial critical path,
  which is what bounds this latency-dominated recurrence.

  PSUM layout per chunk c (shared banks make the fused APs single-tile):
    RUP(c) [128,1024] (2 banks, bufs=3):
       bank A: r0(c) cols 0:256   | r1(c-1) cols 256:512
       bank B: u0(c) cols 512:768 | u1(c-1) cols 768:1024
    OP(c)  [128,512] (1 bank, bufs=2): o0(c) 0:256 | o1(c-1) 256:512
  Banks are seeded by batched x-projection matmuls (L0 biases ride a
  ones-channel appended to X's last K-tile; L1 biases are K=1 matmuls
  against a ones row).

  The state update h' = (1-u)h + u*o is decomposed as m = (u-1)h,
  e = u*o, and the next step's gate pre-activations are accumulated as
  x + (-U)@m + U@e directly in PSUM (pre-negated weight copies), so the
  only work between tanh and the next step's matmuls is one vector op
  (e). h' = e - m itself is computed off the critical path (GpSimd) into
  paired state tiles: hpair(c) slot tl holds [h0(t) | h1(t-8)], written
  by one op and consumed as one operand by the next slot.

  Everything the PE touches is bf16 (fp32 matmuls double-pump the PE);
  PSUM accumulation stays fp32; activations read fp32 PSUM and emit
  bf16. bf16 end-to-end rel err vs fp32 reference = 4.1e-3 (tol 2e-2).
"""

import numpy as np
import ml_dtypes

HID = 128
IN_CH = 271
SEQ = 281
NCLS = 1854
BATCH = 256
NCORES = 8
BL = BATCH // NCORES  # 32 per-core batch
LCH = 8               # timesteps per chunk == layer pipeline lag
G3 = 3 * HID
CW = LCH * BL         # 256: one gate region width
R0, R1, U0, U1 = 0, CW, 2 * CW, 3 * CW
O0, O1 = 0, CW

_CACHE = {}


def _build(seq_t):
    import concourse.bacc as bacc
    import concourse.tile as tile
    import concourse.mybir as mybir
    from contextlib import ExitStack

    fp32 = mybir.dt.float32
    bf16 = mybir.dt.bfloat16
    AF = mybir.ActivationFunctionType
    ALU = mybir.AluOpType

    nch = (seq_t + LCH - 1) // LCH
    chlen = [min(LCH, seq_t - c * LCH) for c in range(nch)]

    nc = bacc.Bacc()
    XT = nc.dram_tensor("XT", [IN_CH, seq_t * BL], bf16, kind="ExternalInput")
    WX0 = nc.dram_tensor("WX0", [IN_CH + 1, G3], bf16, kind="ExternalInput")
    UH0 = nc.dram_tensor("UH0", [HID, G3], bf16, kind="ExternalInput")
    UN0 = nc.dram_tensor("UN0", [HID, 2 * HID], bf16, kind="ExternalInput")
    WX1 = nc.dram_tensor("WX1", [HID, G3], bf16, kind="ExternalInput")
    UH1 = nc.dram_tensor("UH1", [HID, G3], bf16, kind="ExternalInput")
    UN1 = nc.dram_tensor("UN1", [HID, 2 * HID], bf16, kind="ExternalInput")
    B1R = nc.dram_tensor("B1R", [1, G3], bf16, kind="ExternalInput")
    WFC = nc.dram_tensor("WFC", [HID, NCLS], bf16, kind="ExternalInput")
    BFC = nc.dram_tensor("BFC", [1, NCLS], bf16, kind="ExternalInput")
    OUT = nc.dram_tensor("OUT", [BL, NCLS], fp32, kind="ExternalOutput")

    ksz = [128, 128, IN_CH - 256 + 1]  # third tile: 15 channels + ones row

    with tile.TileContext(nc) as tc:
        with ExitStack() as ctx:
            const = ctx.enter_context(tc.tile_pool(name="const", bufs=1))
            hps = ctx.enter_context(tc.tile_pool(name="hps", bufs=3))
            cellp = ctx.enter_context(tc.tile_pool(name="cellp", bufs=8))
            outp = ctx.enter_context(tc.tile_pool(name="outp", bufs=1))
            rup = ctx.enter_context(tc.tile_pool(name="rup", bufs=3, space="PSUM"))
            opp = ctx.enter_context(tc.tile_pool(name="opp", bufs=2, space="PSUM"))

            # ---- constants into SBUF ----
            xt_sb = []
            for k in range(3):
                t_ = const.tile([ksz[k], seq_t * BL], bf16, tag=f"xt{k}")
                c0 = sum(ksz[:k])
                if k < 2:
                    nc.sync.dma_start(out=t_, in_=XT[c0:c0 + ksz[k], :])
                else:
                    # row 15 is the ones-channel carrying the L0 biases
                    nc.vector.memset(t_, 1.0)
                    nc.sync.dma_start(out=t_[0:15, :], in_=XT[256:271, :])
                xt_sb.append(t_)
            wx0_sb = []
            for k in range(3):
                t_ = const.tile([ksz[k], G3], bf16, tag=f"wx0{k}")
                c0 = sum(ksz[:k])
                nc.sync.dma_start(out=t_, in_=WX0[c0:c0 + ksz[k], :])
                wx0_sb.append(t_)

            def sbconst(name, dram, shape, dt=bf16):
                t_ = const.tile(shape, dt, tag=name)
                nc.sync.dma_start(out=t_, in_=dram[:, :])
                return t_

            uh0_sb = sbconst("uh0", UH0, [HID, G3])
            un0_sb = sbconst("un0", UN0, [HID, 2 * HID])
            wx1_sb = sbconst("wx1", WX1, [HID, G3])
            uh1_sb = sbconst("uh1", UH1, [HID, G3])
            un1_sb = sbconst("un1", UN1, [HID, 2 * HID])
            b1_sb = sbconst("b1", B1R, [1, G3])
            wfc_sb = sbconst("wfc", WFC, [HID, NCLS])
            bfc_sb = sbconst("bfc", BFC, [1, NCLS])
            ones_sb = const.tile([1, CW], bf16, tag="ones")
            nc.vector.memset(ones_sb, 1.0)
            h0i = const.tile([HID, BL], bf16, tag="h0i")
            nc.vector.memset(h0i, 0.0)

            rupt = {}
            opt_ = {}
            hpair = {}

            pending = []

            def drain_pending(k=2):
                for _ in range(min(k, len(pending))):
                    pending.pop(0)()

            def phase_l0(c):
                """Chunk c's L0 r/u/o x-projections, split into N=128 pieces
                so a queued piece never blocks a critical cell matmul for
                long, plus the L1 biases for chunk c-1 (whose gates share
                these banks). k==0/h==0 start=True clears each bank and must
                precede every other write to it."""
                n = chlen[c] * BL
                t0 = c * LCH * BL
                ru = rup.tile([HID, 4 * CW], fp32, tag="rup")
                ob = opp.tile([HID, 2 * CW], fp32, tag="opp")
                rupt[c] = ru
                opt_[c] = ob
                nh = (n + 127) // 128
                for g, dst in ((0, ru), (1, ru), (2, ob)):
                    off = (R0, U0, O0)[g]
                    for k in range(3):
                        for h in range(nh):
                            a, bnd = h * 128, min(n, (h + 1) * 128)
                            def mm(g=g, dst=dst, off=off, k=k, a=a, bnd=bnd):
                                nc.tensor.matmul(
                                    dst[:, off + a:off + bnd],
                                    wx0_sb[k][:, g * HID:(g + 1) * HID],
                                    xt_sb[k][:, t0 + a:t0 + bnd],
                                    start=(k == 0 and a == 0), stop=False)
                            pending.append(mm)
                if c > 0:
                    phase_l1_bias(c - 1, False)

            def phase_l1_bias(c1, first):
                """Bias seed for L1 chunk c1 (gates live in chunk c1+1's
                banks). With first=True (past the last L0 chunk) the target
                banks are fresh: allocate and let the bias matmuls clear."""
                n = chlen[c1] * BL
                if first:
                    ru_n = rup.tile([HID, 4 * CW], fp32, tag="rup")
                    ob_n = opp.tile([HID, 2 * CW], fp32, tag="opp")
                    rupt[c1 + 1] = ru_n
                    opt_[c1 + 1] = ob_n
                ru = rupt[c1 + 1]
                ob = opt_[c1 + 1]
                for off, wslice, dst in (
                        (R1, (0, HID), ru), (U1, (HID, 2 * HID), ru),
                        (O1, (2 * HID, G3), ob)):
                    def mmb(off=off, wslice=wslice, dst=dst):
                        nc.tensor.matmul(
                            dst[:, off:off + n], b1_sb[:, wslice[0]:wslice[1]],
                            ones_sb[:, 0:n], start=first, stop=False)
                    pending.append(mmb)

            def emit_next_mms(uh, un, rhs_m, rhs_e, t_next, roff, uoff, which):
                """Accumulate (-U)@m / U@e into step t_next's gate columns.
                which=0 -> the m pair (stop=False), which=1 -> e (stop=True)."""
                cn, sn = divmod(t_next, LCH)
                ru = rupt[cn] if roff == R0 else rupt[cn + 1]
                s = sn * BL
                if which == 0:
                    nc.tensor.matmul(ru[:, roff + s:roff + s + BL],
                                     un[:, 0:HID], rhs_m,
                                     start=False, stop=False)
                    nc.tensor.matmul(ru[:, uoff + s:uoff + s + BL],
                                     un[:, HID:2 * HID], rhs_m,
                                     start=False, stop=False)
                else:
                    nc.tensor.matmul(ru[:, roff + s:roff + s + BL],
                                     uh[:, 0:HID], rhs_e,
                                     start=False, stop=True)
                    nc.tensor.matmul(ru[:, uoff + s:uoff + s + BL],
                                     uh[:, HID:2 * HID], rhs_e,
                                     start=False, stop=True)

            def slot(c, tl, fused, l0_only, hp_prev, h_out):
                """One pipeline slot: layer0 at t=c*8+tl (unless tail),
                layer1 at t-8 (if fused or tail). Operand width W is 64 for
                fused slots, 32 otherwise.

                hp_prev: [128, W] AP of the previous state (pair); h_out:
                [128, W] AP to write the new state (pair)."""
                t = c * LCH + tl
                s = tl * BL
                W = 2 * BL if fused else BL
                nl = 2 if fused else 1
                ru = rupt[c]
                ob = opt_[c]
                if fused:
                    rsrc = ru.rearrange("p (g x) -> p g x", g=4)[
                        :, 0:2, s:s + BL]
                    usrc = ru.rearrange("p (g x) -> p g x", g=4)[
                        :, 2:4, s:s + BL]
                    osrc = ob.rearrange("p (g x) -> p g x", g=2)[
                        :, :, s:s + BL]
                elif l0_only:
                    rsrc = ru[:, R0 + s:R0 + s + BL]
                    usrc = ru[:, U0 + s:U0 + s + BL]
                    osrc = ob[:, O0 + s:O0 + s + BL]
                else:  # tail: layer1 only; gates live in chunk c+1's banks
                    ru = rupt[c + 1]
                    ob = opt_[c + 1]
                    rsrc = ru[:, R1 + s:R1 + s + BL]
                    usrc = ru[:, U1 + s:U1 + s + BL]
                    osrc = ob[:, O1 + s:O1 + s + BL]

                ru_r = cellp.tile([HID, W], bf16, tag="rur")
                ru_u = cellp.tile([HID, W], bf16, tag="ruu")
                if fused:
                    nc.scalar.activation(
                        ru_r.rearrange("p (l x) -> p l x", l=2), rsrc,
                        AF.Sigmoid)
                    nc.scalar.activation(
                        ru_u.rearrange("p (l x) -> p l x", l=2), usrc,
                        AF.Sigmoid)
                else:
                    nc.scalar.activation(ru_r[:, 0:BL], rsrc, AF.Sigmoid)
                    nc.scalar.activation(ru_u[:, 0:BL], usrc, AF.Sigmoid)
                rh = cellp.tile([HID, W], bf16, tag="rh")
                nc.vector.tensor_mul(rh[:, 0:W], ru_r[:, 0:W], hp_prev)
                m = cellp.tile([HID, W], bf16, tag="m")
                nc.vector.scalar_tensor_tensor(
                    m[:, 0:W], ru_u[:, 0:W], 1.0, hp_prev,
                    op0=ALU.subtract, op1=ALU.mult)
                # (-U)@m accumulation into the next step's gate columns
                if l0_only or fused:
                    if t + 1 < seq_t:
                        emit_next_mms(uh0_sb, un0_sb, m[:, 0:BL], None,
                                      t + 1, R0, U0, 0)
                if not l0_only:
                    t1 = t - LCH if fused else t
                    mo = m[:, BL:2 * BL] if fused else m[:, 0:BL]
                    if t1 + 1 < seq_t:
                        emit_next_mms(uh1_sb, un1_sb, mo, None,
                                      t1 + 1, R1, U1, 0)
                drain_pending(1)
                # o-gate matmuls
                if l0_only or fused:
                    nc.tensor.matmul(ob[:, O0 + s:O0 + s + BL],
                                     uh0_sb[:, 2 * HID:G3], rh[:, 0:BL],
                                     start=False, stop=True)
                if not l0_only:
                    rho = rh[:, BL:2 * BL] if fused else rh[:, 0:BL]
                    oo = O1 if (fused or not l0_only) else O0
                    nc.tensor.matmul(ob[:, oo + s:oo + s + BL],
                                     uh1_sb[:, 2 * HID:G3], rho,
                                     start=False, stop=True)
                drain_pending(1)
                o_t = cellp.tile([HID, W], bf16, tag="ot")
                if fused:
                    nc.scalar.activation(
                        o_t.rearrange("p (l x) -> p l x", l=2), osrc, AF.Tanh)
                else:
                    nc.scalar.activation(o_t[:, 0:BL], osrc, AF.Tanh)
                e = cellp.tile([HID, W], bf16, tag="e")
                nc.vector.tensor_mul(e[:, 0:W], ru_u[:, 0:W], o_t[:, 0:W])
                # U@e accumulation into the next step's gate columns
                if l0_only or fused:
                    if t + 1 < seq_t:
                        emit_next_mms(uh0_sb, un0_sb, None, e[:, 0:BL],
                                      t + 1, R0, U0, 1)
                if not l0_only:
                    t1 = t - LCH if fused else t
                    eo = e[:, BL:2 * BL] if fused else e[:, 0:BL]
                    if t1 + 1 < seq_t:
                        emit_next_mms(uh1_sb, un1_sb, None, eo,
                                      t1 + 1, R1, U1, 1)
                drain_pending(1)
                # h' = e - m, off the critical path
                nc.gpsimd.tensor_sub(h_out, e[:, 0:W], m[:, 0:W])
                # incremental L1 x-projection: Wx1 @ h0(t) feeds layer1's
                # step t, which runs LCH slots from now -- far off the
                # critical path, and it removes the batched end-of-chunk
                # projection burst that used to sit right in front of the
                # next chunk's first sigma
                if l0_only or fused:
                    cn1 = t // LCH + 1
                    s1 = (t % LCH) * BL
                    h0new = h_out[:, 0:BL]
                    last = t == 0  # t'=0 gets no m/e matmuls: close its group
                    nc.tensor.matmul(
                        rupt[cn1][:, R1 + s1:R1 + s1 + BL], wx1_sb[:, 0:HID],
                        h0new, start=False, stop=last)
                    nc.tensor.matmul(
                        rupt[cn1][:, U1 + s1:U1 + s1 + BL],
                        wx1_sb[:, HID:2 * HID], h0new, start=False, stop=last)
                    nc.tensor.matmul(
                        opt_[cn1][:, O1 + s1:O1 + s1 + BL],
                        wx1_sb[:, 2 * HID:G3], h0new, start=False, stop=False)
                return h_out

            # ---------- main pipeline ----------
            phase_l0(0)
            phase_l0(1)

            for c in range(nch):
                hp = hps.tile([HID, LCH * 2 * BL], bf16, tag="hpair")
                hpair[c] = hp
                if c == 0:
                    nc.vector.memset(hp, 0.0)
                for tl in range(chlen[c]):
                    if tl == 0:
                        # flush everything queued so far: this chunk's banks
                        # (phase_l0(c) + the L1 biases) must be emitted before
                        # the sigmas and incremental projections that use them
                        drain_pending(len(pending))
                        if c + 2 < nch:
                            phase_l0(c + 2)
                        elif c + 2 == nch:
                            # chunk nch's banks hold only the tail L1 chunk's
                            # gates: bias matmuls allocate and clear them
                            phase_l1_bias(nch - 1, True)
                    t = c * LCH + tl
                    if tl == 0:
                        hp_prev_t = hpair[c - 1] if c > 0 else None
                        pslot = (LCH - 1) * 2 * BL
                    else:
                        hp_prev_t = hp
                        pslot = (tl - 1) * 2 * BL
                    if c == 0:
                        hp_prev = h0i[:, 0:BL] if tl == 0 else \
                            hp_prev_t[:, pslot:pslot + BL]
                        h_out = hp[:, tl * 2 * BL:tl * 2 * BL + BL]
                        slot(c, tl, False, True, hp_prev, h_out)
                    else:
                        hp_prev = hp_prev_t[:, pslot:pslot + 2 * BL]
                        h_out = hp[:, tl * 2 * BL:(tl + 1) * 2 * BL]
                        slot(c, tl, True, False, hp_prev, h_out)

            # ---------- tail: remaining layer1 steps ----------
            # fused slots covered layer1 through t1 = seq_t-1-LCH; the last
            # LCH steps run unfused. h1(t1-1) was written by the fused slot
            # pairing layer0 step t1-1+LCH.
            tp = seq_t - LCH - 1 + LCH  # = seq_t-1: slot of h1(seq_t-LCH-1)
            cp, tlp = divmod(tp, LCH)
            h1_cur = hpair[cp][:, tlp * 2 * BL + BL:(tlp + 1) * 2 * BL]
            for t1 in range(seq_t - LCH, seq_t):
                c1, tl1 = divmod(t1, LCH)
                if tl1 == 0:
                    drain_pending(len(pending))
                h1n = cellp.tile([HID, BL], bf16, tag="h1t")
                slot(c1, tl1, False, False, h1_cur, h1n[:, :])
                h1_cur = h1n[:, :]

            drain_pending(len(pending))

            # ---------- FC ----------
            out_sb = outp.tile([BL, NCLS], fp32, tag="osb")
            nsl = [512, 512, 512, NCLS - 3 * 512]
            for i in range(4):
                n0 = i * 512
                fc = rup.tile([BL, 512], fp32, tag="rup")
                pf = fc[:, 0:nsl[i]]
                nc.tensor.matmul(pf, ones_sb[:, 0:BL], bfc_sb[:, n0:n0 + nsl[i]],
                                 start=True, stop=False)
                nc.tensor.matmul(pf, h1_cur, wfc_sb[:, n0:n0 + nsl[i]],
                                 start=False, stop=True)
                nc.scalar.activation(out_sb[:, n0:n0 + nsl[i]], pf, AF.Identity)
            nc.sync.dma_start(out=OUT[:, :], in_=out_sb)

    nc.finalize()
    return nc


def _prep_consts(inputs):
    bf = ml_dtypes.bfloat16
    Wx0 = np.ascontiguousarray(np.concatenate([
        np.concatenate([inputs["Wr0"][:IN_CH], inputs["Wu0"][:IN_CH],
                        inputs["Wo0"][:IN_CH]], axis=1),
        np.concatenate([inputs["br0"], inputs["bu0"], inputs["bo0"]])[None, :],
    ], axis=0).astype(bf))
    Uh0 = np.concatenate(
        [inputs["Wr0"][IN_CH:], inputs["Wu0"][IN_CH:], inputs["Wo0"][IN_CH:]],
        axis=1).astype(bf)
    Uh1 = np.concatenate(
        [inputs["Wr1"][HID:], inputs["Wu1"][HID:], inputs["Wo1"][HID:]],
        axis=1).astype(bf)
    Un0 = np.ascontiguousarray(-Uh0[:, 0:2 * HID])
    Un1 = np.ascontiguousarray(-Uh1[:, 0:2 * HID])
    Wx1 = np.ascontiguousarray(np.concatenate(
        [inputs["Wr1"][:HID], inputs["Wu1"][:HID], inputs["Wo1"][:HID]],
        axis=1).astype(bf))
    B1R = np.ascontiguousarray(np.concatenate(
        [inputs["br1"], inputs["bu1"], inputs["bo1"]])[None, :].astype(bf))
    WFC = np.ascontiguousarray(inputs["Wfc"].astype(bf))
    BFC = np.ascontiguousarray(inputs["bfc"][None, :].astype(bf))
    return dict(WX0=Wx0, UH0=np.ascontiguousarray(Uh0), UN0=Un0,
                WX1=Wx1, UH1=np.ascontiguousarray(Uh1), UN1=Un1,
                B1R=B1R, WFC=WFC, BFC=BFC)


def kernel(_trace=False, **inputs):
    from concourse.bass_utils import run_bass_kernel_spmd

    seq_t = inputs["X"].shape[2]
    if "nc" not in _CACHE or _CACHE.get("seq_t") != seq_t:
        _CACHE["nc"] = _build(seq_t)
        _CACHE["seq_t"] = seq_t
    nc = _CACHE["nc"]

    consts = _prep_consts(inputs)
    bf = ml_dtypes.bfloat16
    # [B, C, T] -> per-core [C, T, BL] (t-major columns: col = t*BL + b)
    X = inputs["X"].astype(bf)
    in_maps = []
    for c in range(NCORES):
        m = dict(consts)
        xc = X[c * BL:(c + 1) * BL].transpose(1, 2, 0)  # [C, T, BL]
        m["XT"] = np.ascontiguousarray(xc).reshape(IN_CH, seq_t * BL)
        in_maps.append(m)

    res = run_bass_kernel_spmd(nc, in_maps, core_ids=list(range(NCORES)),
                               trace=_trace)
    out = np.concatenate([r["OUT"] for r in res.results], axis=0)
    if _trace:
        _CACHE["last_exec_time_ns"] = res.exec_time_ns
        _CACHE["last_profile"] = res.profile_json
    return out
